# revision 46
# baseline (speedup 1.0000x reference)
"""ALiBi causal self-attention on 8 TRN2 NeuronCores.

Sharding: tensor-parallel over heads for QKV+attention, then AllToAll
to (batch, seq)-parallel for the output projection. Core c owns global
heads (c, 8+c): one steep-slope head whose far-past attention tiles are
statically skipped (ALiBi decay makes them ~e^-18), one shallow head
computed in full — the skip pattern is identical on every core, so the
SPMD graph stays shared. Matmuls run in bf16 (fp32 PSUM accumulate);
ALiBi bias + causal mask are applied in fp32 via one shift-structured
additive map.
"""
import math
import sys

sys.path.insert(0, '/opt/trn_rl_repo')

import numpy as np
import ml_dtypes

B, T, D_MODEL, N_HEADS = 2, 2048, 1024, 16
HEAD_DIM = D_MODEL // N_HEADS      # 64
N_CORES = 8
H_LOC = N_HEADS // N_CORES         # 2 heads per core
KC = D_MODEL // 128                # 8 contraction chunks
NT = T // 512                      # 4 i-chunks of 512
NJ = T // 128                      # 16 j-tiles of 128
TSL = B * T // N_CORES             # 512 output rows per core
BMW = 2432                         # bigmap width
NEG = -1.0e9
# Steep-head (local h0, global heads 0..7, slope >= 2^-4) tile skip:
# a j-tile is dropped when every entry is at least D_SKIP past the
# diagonal. Dropped attention-mass ratio = e^(-slope*D_SKIP) <= e^-8.
D_SKIP = 128


def _jt_range(h, ic):
    """Non-skipped j-tiles for local head h, i-chunk ic."""
    i0 = 512 * ic
    njt = 4 * (ic + 1)
    if h == 0:
        jt_first = max(0, -(-(i0 - 127 - D_SKIP) // 128))
    else:
        jt_first = 0
    return jt_first, njt


def _alibi_slopes(n_heads):
    def pow2(n):
        start = 2.0 ** (-(8.0 / n))
        return [start * (start ** i) for i in range(n)]
    if math.log2(n_heads).is_integer():
        s = pow2(n_heads)
    else:
        cp = 2 ** math.floor(math.log2(n_heads))
        base = pow2(cp)
        extra = pow2(2 * cp)[1::2]
        s = base + extra[: n_heads - cp]
    return np.asarray(s, dtype=np.float32)


_CACHE = {}


def _build():
    if 'nc' in _CACHE:
        return _CACHE['nc']
    import concourse.bacc as bacc
    import concourse.mybir as mybir
    import concourse.tile as tile

    f32 = mybir.dt.float32
    bf16 = mybir.dt.bfloat16
    EXP = mybir.ActivationFunctionType.Exp

    nc = bacc.Bacc('TRN2', target_bir_lowering=False, debug=False,
                   num_devices=N_CORES)

    xT = nc.declare_dram_parameter("xT", [B, D_MODEL, T], bf16, isOutput=False)
    wpack = nc.declare_dram_parameter("wpack", [KC, 128, 384], bf16, isOutput=False)
    woutp = nc.declare_dram_parameter("woutp", [KC, 128, D_MODEL], bf16, isOutput=False)
    bigmap = nc.declare_dram_parameter("bigmap", [128, BMW], f32, isOutput=False)
    jbias_sh = nc.declare_dram_parameter("jbias_sh", [128, 16], f32, isOutput=False)
    out = nc.declare_dram_parameter("out", [TSL, D_MODEL], f32, isOutput=True)

    ones_np = np.ones((128, NJ), dtype=np.float32)
    ident_np = np.tile(np.eye(64, dtype=np.float32), (2, 1))  # [128, 64]
    jl = np.arange(128)[:, None]
    il = np.arange(512)[None, :]
    mstep_np = np.concatenate(
        [np.where(jl + 128 * d <= il, 0.0, NEG).astype(np.float32)
         for d in range(4)], axis=1)                       # [128, 4*512]

    with tile.TileContext(nc) as tc:
        ones_c = nc.inline_tensor(ones_np.astype(ml_dtypes.bfloat16), name="ones16")
        ident_c = nc.inline_tensor(ident_np.astype(ml_dtypes.bfloat16), name="ident64")
        mstep_c = nc.inline_tensor(mstep_np, name="mstep")
        with tc.tile_pool(name="const", bufs=1) as cpool, \
             tc.tile_pool(name="wq", bufs=1) as wqpool, \
             tc.tile_pool(name="xw", bufs=KC) as xwpool, \
             tc.tile_pool(name="kqv", bufs=2) as kqvpool, \
             tc.tile_pool(name="vaug", bufs=4) as vaugpool, \
             tc.tile_pool(name="wide", bufs=2) as widepool, \
             tc.tile_pool(name="expw", bufs=5) as exppool, \
             tc.tile_pool(name="small", bufs=2) as smallpool, \
             tc.tile_pool(name="ctxf", bufs=KC) as ctxfpool, \
             tc.tile_pool(name="wo", bufs=KC) as wopool, \
             tc.tile_pool(name="dram", bufs=1, space="DRAM") as dram, \
             tc.tile_pool(name="psA", bufs=2, space="PSUM") as psA, \
             tc.tile_pool(name="psS", bufs=3, space="PSUM") as psS, \
             tc.tile_pool(name="psC", bufs=3, space="PSUM") as psC:

            # ---- weights first so QKV matmuls can start immediately ----
            wp_ts = []
            for k in range(KC):
                wpt = wqpool.tile([128, 384], bf16, tag=f"wp{k}")
                nc.sync.dma_start(wpt[:], wpack[k])
                wp_ts.append(wpt)

            ident_t = cpool.tile([128, 64], bf16, tag="ident")
            nc.scalar.dma_start(ident_t[:], ident_c.ap())
            a2aH0_in = dram.tile([N_CORES, 64, 512], bf16)
            a2aH0_out = dram.tile([N_CORES, 64, 512], bf16)
            a2aH1_in = dram.tile([N_CORES, 64, 512], bf16)
            a2aH1_out = dram.tile([N_CORES, 64, 512], bf16)
            a2a_ins = [a2aH0_in, a2aH1_in]

            def phase_a(b):
                """QKV projection for batch b -> (ksb, qsb, vaug).

                K^T/Q^T land feature-major for the attention matmuls; V is
                computed directly in [token, dim] orientation (x chunk as the
                stationary operand) so no PE transpose pass is needed."""
                xts = []
                for k in range(KC):
                    xt = xwpool.tile([128, T], bf16, tag="xw")
                    eng = nc.scalar if (b == 0 and k % 2 == 1) else nc.sync
                    eng.dma_start(xt[:], xT[b, k * 128:(k + 1) * 128, :])
                    xts.append(xt)
                ksb = kqvpool.tile([128, T], bf16, tag="ksb")
                qsb = kqvpool.tile([128, T], bf16, tag="qsb")
                for pi, dst in enumerate((ksb, qsb)):
                    for tcb in range(NT):
                        ps = psA.tile([128, 512], f32, tag="psA")
                        for k in range(KC):
                            nc.tensor.matmul(
                                ps[:],
                                wp_ts[k][:, pi * 128:(pi + 1) * 128],
                                xts[k][:, tcb * 512:(tcb + 1) * 512],
                                start=(k == 0), stop=(k == KC - 1))
                        nc.any.tensor_copy(dst[:, tcb * 512:(tcb + 1) * 512], ps[:])
                vaug = []
                for h in range(H_LOC):
                    va = vaugpool.tile([128, NJ * 65], bf16, tag="vaug")
                    va_ones = va[:].rearrange("p (c w) -> p c w", w=65)[:, :, 64:65]
                    nc.sync.dma_start(va_ones, ones_c.ap())
                    vaug.append(va)
                # V^T via one efficient N=512 matmul pack, then bf16 PE
                # transposes (1 cyc/row) into [token, dim] layout
                vtsb = kqvpool.tile([128, T], bf16, tag="vtsb")
                for tcb in range(NT):
                    ps = psA.tile([128, 512], f32, tag="psA")
                    for k in range(KC):
                        nc.tensor.matmul(
                            ps[:],
                            wp_ts[k][:, 256:384],
                            xts[k][:, tcb * 512:(tcb + 1) * 512],
                            start=(k == 0), stop=(k == KC - 1))
                    nc.any.tensor_copy(vtsb[:, tcb * 512:(tcb + 1) * 512], ps[:])
                for h in range(H_LOC):
                    for jc in range(NJ):
                        pv = psA.tile([128, 64], bf16, tag="psA")
                        nc.tensor.transpose(
                            pv[:],
                            vtsb[64 * h:64 * h + 64, jc * 128:(jc + 1) * 128],
                            ident_t[64 * h:64 * h + 64, :])
                        nc.any.tensor_copy(
                            vaug[h][:, jc * 65:jc * 65 + 64], pv[:])
                return ksb, qsb, vaug

            def attention(b, h, ksb, qsb, vaug):
                """Attention for local head h of batch b; ships ctx to a2a_in."""
                jt_lists = {ic: _jt_range(h, ic) for ic in range(NT)}

                def normalize(ic, pctx):
                    rcin = smallpool.tile([1, 512], f32, tag="rcin")
                    nc.any.tensor_copy(rcin[:], pctx[64:65, :])
                    rc = smallpool.tile([1, 512], f32, tag="rc")
                    nc.vector.reciprocal_approx_fast(rc[:], rcin[:])
                    rcb = smallpool.tile([64, 512], f32, tag="rcb")
                    nc.gpsimd.partition_broadcast(rcb[:], rc[:])
                    ctxn = smallpool.tile([64, 512], bf16, tag="ctxn")
                    nc.vector.tensor_mul(ctxn[:], pctx[0:64, :], rcb[:])
                    s = b * NT + ic
                    nc.sync.dma_start(a2a_ins[h][s], ctxn[:])

                if h == 0:
                    # steep head, jt-outer with wide exp over short spans
                    pctxs = []
                    for _ic in range(NT):
                        pctx = psC.tile([65, 512], f32, tag="psC")
                        pctxs.append(pctx)
                    for jt in range(NJ):
                        j0 = 128 * jt
                        ics = [ic for ic in range(NT)
                               if jt_lists[ic][0] <= jt < jt_lists[ic][1]]
                        if not ics:
                            continue
                        span_lo, span_hi = 512 * ics[0], 512 * (ics[-1] + 1)
                        mw = widepool.tile([128, T], f32, tag="mw")
                        for ic in ics:
                            i0 = 512 * ic
                            ps = psS.tile([128, 512], f32, tag="psS")
                            nc.tensor.matmul(
                                ps[:],
                                ksb[64 * h:64 * h + 64, j0:j0 + 128],
                                qsb[64 * h:64 * h + 64, i0:i0 + 512],
                                start=True, stop=True)
                            off = i0 - j0 + 384
                            nc.vector.tensor_add(
                                mw[:, i0:i0 + 512], ps[:], bm_t[:, off:off + 512])
                        ew = exppool.tile([128, T], bf16, tag="ew")
                        nc.scalar.activation(
                            ew[:, span_lo:span_hi], mw[:, span_lo:span_hi], EXP)
                        for ic in ics:
                            i0 = 512 * ic
                            nc.tensor.matmul(
                                pctxs[ic][:],
                                vaug[h][:, jt * 65:(jt + 1) * 65],
                                ew[:, i0:i0 + 512],
                                start=(jt == jt_lists[ic][0]),
                                stop=(jt == jt_lists[ic][1] - 1))
                    for ic in range(NT):
                        normalize(ic, pctxs[ic])
                else:
                    # shallow head, ic-outer: one live ctx accumulator
                    for ic in range(NT):
                        i0 = 512 * ic
                        jtf, jtl = jt_lists[ic]
                        pctx = psC.tile([65, 512], f32, tag="psC")
                        for jt in range(jtf, jtl):
                            j0 = 128 * jt
                            d = j0 - i0
                            dc = d // 128 + 12
                            ps = psS.tile([128, 512], f32, tag="psS")
                            nc.tensor.matmul(
                                ps[:],
                                ksb[64 * h:64 * h + 64, j0:j0 + 128],
                                qsb[64 * h:64 * h + 64, i0:i0 + 512],
                                start=True, stop=True)
                            es = exppool.tile([128, 512], bf16, tag="es")
                            if d >= 0:
                                msk = widepool.tile([128, 512], f32, tag="msk")
                                nc.vector.tensor_add(
                                    msk[:], ps[:],
                                    ms_t[:, (d // 128) * 512:(d // 128 + 1) * 512])
                                src_ap = msk[:]
                            else:
                                src_ap = ps[:]
                            nc.scalar.activation(
                                es[:], src_ap, EXP, bias=jb_t[:, dc:dc + 1])
                            nc.tensor.matmul(
                                pctx[:],
                                vaug[h][:, jt * 65:(jt + 1) * 65],
                                es[:],
                                start=(jt == jtf), stop=(jt == jtl - 1))
                        normalize(ic, pctx)

            # ---- emission order tuned for overlap ----
            k0, q0, va0 = phase_a(0)
            bm_t = cpool.tile([128, BMW], f32, tag="bm")
            nc.scalar.dma_start(bm_t[:], bigmap[:])
            jb_t = cpool.tile([128, 16], f32, tag="jb")
            nc.scalar.dma_start(jb_t[:], jbias_sh[:])
            ms_t = cpool.tile([128, 4 * 512], f32, tag="ms")
            nc.scalar.dma_start(ms_t[:], mstep_c.ap())
            attention(0, 0, k0, q0, va0)
            k1, q1, va1 = phase_a(1)        # fills PE while b0 attention runs
            attention(1, 0, k1, q1, va1)
            # h0 context for both batches is ready: overlap its exchange
            # with the whole h1 attention pass
            nc.gpsimd.collective_compute(
                "AllToAll", mybir.AluOpType.bypass,
                replica_groups=[list(range(N_CORES))],
                ins=[a2aH0_in.opt()], outs=[a2aH0_out.opt()])
            attention(0, 1, k0, q0, va0)
            wouts = []
            for k in range(KC):
                wo = wopool.tile([128, D_MODEL], bf16, tag="wo")
                nc.sync.dma_start(wo[:], woutp[k])
                wouts.append(wo)
            attention(1, 1, k1, q1, va1)
            # h0 context loads AFTER the last ctxn store so they cannot
            # head-of-line-block the sync queue while A2A#1 is in flight;
            # they and the h0 projection half then run during A2A#2
            cfh0 = []
            for p in range(4):
                cf0 = ctxfpool.tile([128, 512], bf16, tag="cfh0")
                nc.sync.dma_start(cf0[0:64, :], a2aH0_out[2 * p])
                nc.sync.dma_start(cf0[64:128, :], a2aH0_out[2 * p + 1])
                cfh0.append(cf0)
            outh0 = []
            for tt in range(4):
                for dh in range(2):
                    poh = psS.tile([128, 512], f32, tag="psS")
                    for p in range(4):
                        nc.tensor.matmul(
                            poh[:],
                            cfh0[p][:, tt * 128:(tt + 1) * 128],
                            wouts[p][:, dh * 512:(dh + 1) * 512],
                            start=(p == 0), stop=(p == 3))
                    oh = ctxfpool.tile([128, 512], bf16, tag="outh0")
                    nc.any.tensor_copy(oh[:], poh[:])
                    outh0.append(oh)

            nc.gpsimd.collective_compute(
                "AllToAll", mybir.AluOpType.bypass,
                replica_groups=[list(range(N_CORES))],
                ins=[a2aH1_in.opt()], outs=[a2aH1_out.opt()])

            # ---- phase C: h1 half + combine ----
            cfh1 = []
            for p in range(4):
                cf1 = ctxfpool.tile([128, 512], bf16, tag="cfh1")
                nc.sync.dma_start(cf1[0:64, :], a2aH1_out[2 * p])
                nc.sync.dma_start(cf1[64:128, :], a2aH1_out[2 * p + 1])
                cfh1.append(cf1)
            for tt in range(4):
                for dh in range(2):
                    po = psS.tile([128, 512], f32, tag="psS")
                    for p in range(4):
                        nc.tensor.matmul(
                            po[:],
                            cfh1[p][:, tt * 128:(tt + 1) * 128],
                            wouts[4 + p][:, dh * 512:(dh + 1) * 512],
                            start=(p == 0), stop=(p == 3))
                    osb = smallpool.tile([128, 512], f32, tag="osb")
                    nc.vector.tensor_add(osb[:], po[:], outh0[tt * 2 + dh][:])
                    nc.sync.dma_start(
                        out[tt * 128:(tt + 1) * 128, dh * 512:(dh + 1) * 512],
                        osb[:])
    nc.compile()
    _CACHE['nc'] = nc
    return nc


def _prep_inputs(x, Wqkv, Wout):
    slopes = _alibi_slopes(N_HEADS)
    scale = 1.0 / math.sqrt(HEAD_DIM)
    xT = np.ascontiguousarray(x.transpose(0, 2, 1)).astype(ml_dtypes.bfloat16)
    Wq = Wqkv[:, 0:D_MODEL]
    Wk = Wqkv[:, D_MODEL:2 * D_MODEL]
    Wv = Wqkv[:, 2 * D_MODEL:3 * D_MODEL]

    # core c owns global heads (c, 8+c). Phase C consumes rank-PAIRED
    # chunks: chunks 0..3 = h0 rows of rank pairs (0,1),(2,3),(4,5),(6,7);
    # chunks 4..7 = the h1 rows of the same pairs.
    perm = []
    for p in range(4):
        for r in (2 * p, 2 * p + 1):
            perm.extend(range(r * HEAD_DIM, (r + 1) * HEAD_DIM))
    for p in range(4):
        for r in (2 * p, 2 * p + 1):
            hg = 8 + r
            perm.extend(range(hg * HEAD_DIM, (hg + 1) * HEAD_DIM))
    woutp = np.ascontiguousarray(
        Wout[perm].reshape(KC, 128, D_MODEL)).astype(ml_dtypes.bfloat16)

    jloc = np.arange(128, dtype=np.float64)[:, None]
    col = np.arange(BMW, dtype=np.float64)[None, :]
    rel = jloc - (col - 384.0)          # j_abs - i_abs at these coords
    keep = rel <= 0.0

    in_maps = []
    for c in range(N_CORES):
        heads = (c, 8 + c)
        cols = []
        for W, s in ((Wk, 1.0), (Wq, scale), (Wv, 1.0)):
            for hg in heads:
                cols.append(W[:, hg * HEAD_DIM:(hg + 1) * HEAD_DIM] * s)
        wp = np.concatenate(cols, axis=1)                    # [1024, 384]
        wpack = np.ascontiguousarray(
            wp.reshape(KC, 128, 384)).astype(ml_dtypes.bfloat16)

        sl0 = float(slopes[heads[0]])
        bm = np.where(keep, sl0 * rel, NEG).astype(np.float32)
        sl1 = float(slopes[heads[1]])
        jb = np.zeros((128, 16), np.float32)
        for dc in range(16):
            dd = 128 * (dc - 12)
            jb[:, dc] = sl1 * (np.arange(128) + dd)
        in_maps.append({
            "xT": xT, "wpack": wpack, "woutp": woutp, "bigmap": bm,
            "jbias_sh": jb,
        })
    return in_maps


def kernel(x, Wqkv, Wout):
    from concourse.bass_utils import run_bass_kernel_spmd
    nc = _build()
    in_maps = _prep_inputs(np.asarray(x, dtype=np.float32),
                           np.asarray(Wqkv, dtype=np.float32),
                           np.asarray(Wout, dtype=np.float32))
    res = run_bass_kernel_spmd(nc, in_maps, list(range(N_CORES)))
    slices = [res.results[c]["out"] for c in range(N_CORES)]
    return np.concatenate(slices, axis=0).reshape(B, T, D_MODEL)


# revision 48
# speedup vs baseline: 1.1721x; 1.1721x over previous
"""ALiBi causal self-attention on 8 TRN2 NeuronCores.

Sharding: tensor-parallel over heads for QKV+attention, then AllToAll
to (batch, seq)-parallel for the output projection. Core c owns global
heads (c, 8+c): one steep-slope head whose far-past attention tiles are
statically skipped (ALiBi decay makes them ~e^-18), one shallow head
computed in full — the skip pattern is identical on every core, so the
SPMD graph stays shared. Matmuls run in bf16 (fp32 PSUM accumulate);
ALiBi bias + causal mask are applied in fp32 via one shift-structured
additive map.
"""
import math
import sys

sys.path.insert(0, '/opt/trn_rl_repo')

import numpy as np
import ml_dtypes

B, T, D_MODEL, N_HEADS = 2, 2048, 1024, 16
HEAD_DIM = D_MODEL // N_HEADS      # 64
N_CORES = 8
H_LOC = N_HEADS // N_CORES         # 2 heads per core
KC = D_MODEL // 128                # 8 contraction chunks
NT = T // 512                      # 4 i-chunks of 512
NJ = T // 128                      # 16 j-tiles of 128
TSL = B * T // N_CORES             # 512 output rows per core
BMW = 2432                         # bigmap width
NEG = -1.0e9
# Steep-head (local h0, global heads 0..7, slope >= 2^-4) tile skip:
# a j-tile is dropped when every entry is at least D_SKIP past the
# diagonal. Dropped attention-mass ratio = e^(-slope*D_SKIP) <= e^-8.
D_SKIP = 128


def _jt_range(h, ic):
    """Non-skipped j-tiles for local head h, i-chunk ic."""
    i0 = 512 * ic
    njt = 4 * (ic + 1)
    if h == 0:
        jt_first = max(0, -(-(i0 - 127 - D_SKIP) // 128))
    else:
        jt_first = 0
    return jt_first, njt


def _alibi_slopes(n_heads):
    def pow2(n):
        start = 2.0 ** (-(8.0 / n))
        return [start * (start ** i) for i in range(n)]
    if math.log2(n_heads).is_integer():
        s = pow2(n_heads)
    else:
        cp = 2 ** math.floor(math.log2(n_heads))
        base = pow2(cp)
        extra = pow2(2 * cp)[1::2]
        s = base + extra[: n_heads - cp]
    return np.asarray(s, dtype=np.float32)


_CACHE = {}


def _build():
    if 'nc' in _CACHE:
        return _CACHE['nc']
    import concourse.bacc as bacc
    import concourse.mybir as mybir
    import concourse.tile as tile

    f32 = mybir.dt.float32
    bf16 = mybir.dt.bfloat16
    EXP = mybir.ActivationFunctionType.Exp

    nc = bacc.Bacc('TRN2', target_bir_lowering=False, debug=False,
                   num_devices=N_CORES)

    xT = nc.declare_dram_parameter("xT", [B, D_MODEL, T], bf16, isOutput=False)
    wpack = nc.declare_dram_parameter("wpack", [KC, 128, 384], bf16, isOutput=False)
    woutp = nc.declare_dram_parameter("woutp", [KC, 128, D_MODEL], bf16, isOutput=False)
    bigmap = nc.declare_dram_parameter("bigmap", [128, BMW], f32, isOutput=False)
    jbias_sh = nc.declare_dram_parameter("jbias_sh", [128, 16], f32, isOutput=False)
    out = nc.declare_dram_parameter("out", [TSL, D_MODEL], f32, isOutput=True)

    ones_np = np.ones((128, NJ), dtype=np.float32)
    ident_np = np.tile(np.eye(64, dtype=np.float32), (2, 1))  # [128, 64]
    jl = np.arange(128)[:, None]
    il = np.arange(512)[None, :]
    mstep_np = np.concatenate(
        [np.where(jl + 128 * d <= il, 0.0, NEG).astype(np.float32)
         for d in range(4)], axis=1)                       # [128, 4*512]

    with tile.TileContext(nc) as tc:
        ones_c = nc.inline_tensor(ones_np.astype(ml_dtypes.bfloat16), name="ones16")
        ident_c = nc.inline_tensor(ident_np.astype(ml_dtypes.bfloat16), name="ident64")
        mstep_c = nc.inline_tensor(mstep_np, name="mstep")
        with tc.tile_pool(name="const", bufs=1) as cpool, \
             tc.tile_pool(name="wq", bufs=1) as wqpool, \
             tc.tile_pool(name="xw", bufs=KC) as xwpool, \
             tc.tile_pool(name="kqv", bufs=2) as kqvpool, \
             tc.tile_pool(name="vaug", bufs=4) as vaugpool, \
             tc.tile_pool(name="wide", bufs=2) as widepool, \
             tc.tile_pool(name="expw", bufs=5) as exppool, \
             tc.tile_pool(name="small", bufs=2) as smallpool, \
             tc.tile_pool(name="ctxf", bufs=KC) as ctxfpool, \
             tc.tile_pool(name="wo", bufs=KC) as wopool, \
             tc.tile_pool(name="dram", bufs=1, space="DRAM") as dram, \
             tc.tile_pool(name="psA", bufs=2, space="PSUM") as psA, \
             tc.tile_pool(name="psS", bufs=4, space="PSUM") as psS, \
             tc.tile_pool(name="psC", bufs=2, space="PSUM") as psC:

            # ---- weights first so QKV matmuls can start immediately ----
            wp_ts = []
            for k in range(KC):
                wpt = wqpool.tile([128, 384], bf16, tag=f"wp{k}")
                nc.sync.dma_start(wpt[:], wpack[k])
                wp_ts.append(wpt)

            ident_t = cpool.tile([128, 64], bf16, tag="ident")
            nc.scalar.dma_start(ident_t[:], ident_c.ap())
            a2aH0_in = dram.tile([N_CORES, 64, 512], bf16)
            a2aH0_out = dram.tile([N_CORES, 64, 512], bf16)
            a2aH1_in = dram.tile([N_CORES, 64, 512], bf16)
            a2aH1_out = dram.tile([N_CORES, 64, 512], bf16)
            a2a_ins = [a2aH0_in, a2aH1_in]

            def phase_a(b):
                """QKV projection for batch b -> (ksb, qsb, vaug).

                K^T/Q^T land feature-major for the attention matmuls; V is
                computed directly in [token, dim] orientation (x chunk as the
                stationary operand) so no PE transpose pass is needed."""
                xts = []
                for k in range(KC):
                    xt = xwpool.tile([128, T], bf16, tag="xw")
                    eng = nc.scalar if (b == 0 and k % 2 == 1) else nc.sync
                    eng.dma_start(xt[:], xT[b, k * 128:(k + 1) * 128, :])
                    xts.append(xt)
                ksb = kqvpool.tile([128, T], bf16, tag="ksb")
                qsb = kqvpool.tile([128, T], bf16, tag="qsb")
                for pi, dst in enumerate((ksb, qsb)):
                    for tcb in range(NT):
                        ps = psA.tile([128, 512], f32, tag="psA")
                        for k in range(KC):
                            nc.tensor.matmul(
                                ps[:],
                                wp_ts[k][:, pi * 128:(pi + 1) * 128],
                                xts[k][:, tcb * 512:(tcb + 1) * 512],
                                start=(k == 0), stop=(k == KC - 1))
                        nc.any.tensor_copy(dst[:, tcb * 512:(tcb + 1) * 512], ps[:])
                vaug = []
                for h in range(H_LOC):
                    va = vaugpool.tile([128, NJ * 65], bf16, tag="vaug")
                    va_ones = va[:].rearrange("p (c w) -> p c w", w=65)[:, :, 64:65]
                    nc.sync.dma_start(va_ones, ones_c.ap())
                    vaug.append(va)
                # V^T via one efficient N=512 matmul pack, then bf16 PE
                # transposes (1 cyc/row) into [token, dim] layout
                vtsb = kqvpool.tile([128, T], bf16, tag="vtsb")
                for tcb in range(NT):
                    ps = psA.tile([128, 512], f32, tag="psA")
                    for k in range(KC):
                        nc.tensor.matmul(
                            ps[:],
                            wp_ts[k][:, 256:384],
                            xts[k][:, tcb * 512:(tcb + 1) * 512],
                            start=(k == 0), stop=(k == KC - 1))
                    nc.any.tensor_copy(vtsb[:, tcb * 512:(tcb + 1) * 512], ps[:])
                for h in range(H_LOC):
                    for jc in range(NJ):
                        pv = psA.tile([128, 64], bf16, tag="psA")
                        nc.tensor.transpose(
                            pv[:],
                            vtsb[64 * h:64 * h + 64, jc * 128:(jc + 1) * 128],
                            ident_t[64 * h:64 * h + 64, :])
                        nc.any.tensor_copy(
                            vaug[h][:, jc * 65:jc * 65 + 64], pv[:])
                return ksb, qsb, vaug

            def attention(b, h, ksb, qsb, vaug):
                """Attention for local head h of batch b; ships ctx to a2a_in."""
                jt_lists = {ic: _jt_range(h, ic) for ic in range(NT)}

                def normalize(ic, pctx):
                    rcin = smallpool.tile([1, 512], f32, tag="rcin")
                    nc.any.tensor_copy(rcin[:], pctx[64:65, :])
                    rc = smallpool.tile([1, 512], f32, tag="rc")
                    nc.vector.reciprocal_approx_fast(rc[:], rcin[:])
                    rcb = smallpool.tile([64, 512], f32, tag="rcb")
                    nc.gpsimd.partition_broadcast(rcb[:], rc[:])
                    ctxn = smallpool.tile([64, 512], bf16, tag="ctxn")
                    nc.vector.tensor_mul(ctxn[:], pctx[0:64, :], rcb[:])
                    s = b * NT + ic
                    nc.sync.dma_start(a2a_ins[h][s], ctxn[:])

                if h == 0:
                    # steep head, jt-outer with wide exp over short spans
                    pctxs = []
                    for _ic in range(NT):
                        pctx = psC.tile([65, 512], f32, tag="psC")
                        pctxs.append(pctx)
                    for jt in range(NJ):
                        j0 = 128 * jt
                        ics = [ic for ic in range(NT)
                               if jt_lists[ic][0] <= jt < jt_lists[ic][1]]
                        if not ics:
                            continue
                        span_lo, span_hi = 512 * ics[0], 512 * (ics[-1] + 1)
                        mw = widepool.tile([128, T], f32, tag="mw")
                        for ic in ics:
                            i0 = 512 * ic
                            ps = psS.tile([128, 512], f32, tag="psS")
                            nc.tensor.matmul(
                                ps[:],
                                ksb[64 * h:64 * h + 64, j0:j0 + 128],
                                qsb[64 * h:64 * h + 64, i0:i0 + 512],
                                start=True, stop=True)
                            off = i0 - j0 + 384
                            nc.vector.tensor_add(
                                mw[:, i0:i0 + 512], ps[:], bm_t[:, off:off + 512])
                        ew = exppool.tile([128, T], bf16, tag="ew")
                        nc.scalar.activation(
                            ew[:, span_lo:span_hi], mw[:, span_lo:span_hi], EXP)
                        for ic in ics:
                            i0 = 512 * ic
                            nc.tensor.matmul(
                                pctxs[ic][:],
                                vaug[h][:, jt * 65:(jt + 1) * 65],
                                ew[:, i0:i0 + 512],
                                start=(jt == jt_lists[ic][0]),
                                stop=(jt == jt_lists[ic][1] - 1))
                    for ic in range(NT):
                        normalize(ic, pctxs[ic])
                else:
                    # shallow head, ic-outer: one live ctx accumulator
                    for ic in range(NT):
                        i0 = 512 * ic
                        jtf, jtl = jt_lists[ic]
                        pctx = psC.tile([65, 512], f32, tag="psC")
                        for jt in range(jtf, jtl):
                            j0 = 128 * jt
                            d = j0 - i0
                            dc = d // 128 + 12
                            ps = psS.tile([128, 512], f32, tag="psS")
                            nc.tensor.matmul(
                                ps[:],
                                ksb[64 * h:64 * h + 64, j0:j0 + 128],
                                qsb[64 * h:64 * h + 64, i0:i0 + 512],
                                start=True, stop=True)
                            es = exppool.tile([128, 512], bf16, tag="es")
                            if d >= 0:
                                msk = widepool.tile([128, 512], f32, tag="msk")
                                nc.vector.tensor_add(
                                    msk[:], ps[:],
                                    ms_t[:, (d // 128) * 512:(d // 128 + 1) * 512])
                                src_ap = msk[:]
                            else:
                                src_ap = ps[:]
                            nc.scalar.activation(
                                es[:], src_ap, EXP, bias=jb_t[:, dc:dc + 1])
                            nc.tensor.matmul(
                                pctx[:],
                                vaug[h][:, jt * 65:(jt + 1) * 65],
                                es[:],
                                start=(jt == jtf), stop=(jt == jtl - 1))
                        normalize(ic, pctx)

            # ---- emission order tuned for overlap ----
            k0, q0, va0 = phase_a(0)
            bm_t = cpool.tile([128, BMW], f32, tag="bm")
            nc.scalar.dma_start(bm_t[:], bigmap[:])
            jb_t = cpool.tile([128, 16], f32, tag="jb")
            nc.scalar.dma_start(jb_t[:], jbias_sh[:])
            ms_t = cpool.tile([128, 4 * 512], f32, tag="ms")
            nc.scalar.dma_start(ms_t[:], mstep_c.ap())
            attention(0, 0, k0, q0, va0)
            k1, q1, va1 = phase_a(1)        # fills PE while b0 attention runs
            attention(1, 0, k1, q1, va1)
            # h0 context for both batches is ready: overlap its exchange
            # with the whole h1 attention pass
            nc.gpsimd.collective_compute(
                "AllToAll", mybir.AluOpType.bypass,
                replica_groups=[list(range(N_CORES))],
                ins=[a2aH0_in.opt()], outs=[a2aH0_out.opt()])
            attention(0, 1, k0, q0, va0)
            wouts = []
            for k in range(KC):
                wo = wopool.tile([128, D_MODEL], bf16, tag="wo")
                nc.sync.dma_start(wo[:], woutp[k])
                wouts.append(wo)
            attention(1, 1, k1, q1, va1)
            # h0 context loads AFTER the last ctxn store so they cannot
            # head-of-line-block the sync queue while A2A#1 is in flight;
            # they and the h0 projection half then run during A2A#2
            cfh0 = []
            for p in range(4):
                cf0 = ctxfpool.tile([128, 512], bf16, tag="cfh0")
                nc.sync.dma_start(cf0[0:64, :], a2aH0_out[2 * p])
                nc.sync.dma_start(cf0[64:128, :], a2aH0_out[2 * p + 1])
                cfh0.append(cf0)
            outh0 = []
            for tt in range(4):
                for dh in range(2):
                    poh = psS.tile([128, 512], f32, tag="psS")
                    for p in range(4):
                        nc.tensor.matmul(
                            poh[:],
                            cfh0[p][:, tt * 128:(tt + 1) * 128],
                            wouts[p][:, dh * 512:(dh + 1) * 512],
                            start=(p == 0), stop=(p == 3))
                    oh = ctxfpool.tile([128, 512], bf16, tag="outh0")
                    nc.any.tensor_copy(oh[:], poh[:])
                    outh0.append(oh)

            nc.gpsimd.collective_compute(
                "AllToAll", mybir.AluOpType.bypass,
                replica_groups=[list(range(N_CORES))],
                ins=[a2aH1_in.opt()], outs=[a2aH1_out.opt()])

            # ---- phase C: h1 half + combine ----
            cfh1 = []
            for p in range(4):
                cf1 = ctxfpool.tile([128, 512], bf16, tag="cfh1")
                nc.sync.dma_start(cf1[0:64, :], a2aH1_out[2 * p])
                nc.sync.dma_start(cf1[64:128, :], a2aH1_out[2 * p + 1])
                cfh1.append(cf1)
            for tt in range(4):
                for dh in range(2):
                    po = psS.tile([128, 512], f32, tag="psS")
                    for p in range(4):
                        nc.tensor.matmul(
                            po[:],
                            cfh1[p][:, tt * 128:(tt + 1) * 128],
                            wouts[4 + p][:, dh * 512:(dh + 1) * 512],
                            start=(p == 0), stop=(p == 3))
                    osb = smallpool.tile([128, 512], f32, tag="osb")
                    nc.vector.tensor_add(osb[:], po[:], outh0[tt * 2 + dh][:])
                    nc.sync.dma_start(
                        out[tt * 128:(tt + 1) * 128, dh * 512:(dh + 1) * 512],
                        osb[:])
    nc.compile()
    _CACHE['nc'] = nc
    return nc


def _prep_inputs(x, Wqkv, Wout):
    slopes = _alibi_slopes(N_HEADS)
    scale = 1.0 / math.sqrt(HEAD_DIM)
    xT = np.ascontiguousarray(x.transpose(0, 2, 1)).astype(ml_dtypes.bfloat16)
    Wq = Wqkv[:, 0:D_MODEL]
    Wk = Wqkv[:, D_MODEL:2 * D_MODEL]
    Wv = Wqkv[:, 2 * D_MODEL:3 * D_MODEL]

    # core c owns global heads (c, 8+c). Phase C consumes rank-PAIRED
    # chunks: chunks 0..3 = h0 rows of rank pairs (0,1),(2,3),(4,5),(6,7);
    # chunks 4..7 = the h1 rows of the same pairs.
    perm = []
    for p in range(4):
        for r in (2 * p, 2 * p + 1):
            perm.extend(range(r * HEAD_DIM, (r + 1) * HEAD_DIM))
    for p in range(4):
        for r in (2 * p, 2 * p + 1):
            hg = 8 + r
            perm.extend(range(hg * HEAD_DIM, (hg + 1) * HEAD_DIM))
    woutp = np.ascontiguousarray(
        Wout[perm].reshape(KC, 128, D_MODEL)).astype(ml_dtypes.bfloat16)

    jloc = np.arange(128, dtype=np.float64)[:, None]
    col = np.arange(BMW, dtype=np.float64)[None, :]
    rel = jloc - (col - 384.0)          # j_abs - i_abs at these coords
    keep = rel <= 0.0

    in_maps = []
    for c in range(N_CORES):
        heads = (c, 8 + c)
        cols = []
        for W, s in ((Wk, 1.0), (Wq, scale), (Wv, 1.0)):
            for hg in heads:
                cols.append(W[:, hg * HEAD_DIM:(hg + 1) * HEAD_DIM] * s)
        wp = np.concatenate(cols, axis=1)                    # [1024, 384]
        wpack = np.ascontiguousarray(
            wp.reshape(KC, 128, 384)).astype(ml_dtypes.bfloat16)

        sl0 = float(slopes[heads[0]])
        bm = np.where(keep, sl0 * rel, NEG).astype(np.float32)
        sl1 = float(slopes[heads[1]])
        jb = np.zeros((128, 16), np.float32)
        for dc in range(16):
            dd = 128 * (dc - 12)
            jb[:, dc] = sl1 * (np.arange(128) + dd)
        in_maps.append({
            "xT": xT, "wpack": wpack, "woutp": woutp, "bigmap": bm,
            "jbias_sh": jb,
        })
    return in_maps


def kernel(x, Wqkv, Wout):
    import time
    from concourse.bass_utils import run_bass_kernel_spmd
    nc = _build()
    in_maps = _prep_inputs(np.asarray(x, dtype=np.float32),
                           np.asarray(Wqkv, dtype=np.float32),
                           np.asarray(Wout, dtype=np.float32))
    # A crashed earlier process can leave the accelerator wedged
    # (NRT_EXEC_UNIT_UNRECOVERABLE); a plain re-run clears it.
    last_err = None
    for attempt in range(3):
        try:
            res = run_bass_kernel_spmd(nc, in_maps, list(range(N_CORES)))
            break
        except Exception as e:
            last_err = e
            if attempt == 2:
                raise
            time.sleep(3.0)
    slices = [res.results[c]["out"] for c in range(N_CORES)]
    return np.concatenate(slices, axis=0).reshape(B, T, D_MODEL)
